# revision 13
# baseline (speedup 1.0000x reference)
"""Segment-mean GNN aggregation (MeanAggregator) on 8 TRN2 NeuronCores.

out[v] = mean over edges (u -> v) of x[u], zeros for isolated nodes.

Strategy: shard destination nodes across the 8 cores (12500 each) and
replicate x (stored fp16) in every core's DRAM. The host partitions edges
by dst owner, sorts by dst, and packs them into 128-edge chunks grouped
by 128-dst "groups". Because dma_gather (the fast SWDGE gather) takes
int16 indices, x is split into 4 banks of 25000 rows and each chunk's
edges come from a single bank; the per-(group, bank) chunk counts are
maxed over cores so one SPMD program fits all 8 cores.

Device pipeline per core:
  - dma_gather ops of up to 8 chunks (1024 indices) pull source rows into
    SBUF [128 edges x nch x 128 feat] fp16 tiles; the 4 banks ride the 4
    SWDGE queues so all four Q7 pairs generate descriptors concurrently.
  - VectorE builds an exact one-hot S[e, s] = (slot[e] == s) in fp16 per
    chunk (tensor_scalar is_equal against a constant iota row); padding
    slots are -1 and match nothing.
  - TensorE accumulates S.T @ E into PSUM [128 dst x 128 feat] per group.
  - ScalarE copies PSUM to SBUF scaled by fp32 1/max(deg,1) (activation
    with per-partition scale), and the rows are DMA'd to the output.
"""

import math
from contextlib import ExitStack

import numpy as np

import concourse.tile as tile
from concourse import bacc, mybir
from concourse.bass_utils import run_bass_kernel_spmd

N_NODES = 100000
N_FEAT = 128
N_CORES = 8
NODES_PER_CORE = N_NODES // N_CORES  # 12500
P = 128
N_GROUPS = math.ceil(NODES_PER_CORE / P)  # 98
N_BANKS = 4
BANK = N_NODES // N_BANKS  # 25000 rows per bank (int16-indexable)
OP_CHUNKS = 8  # chunks per dma_gather op (1024 indices; single-packet safe)

_compiled_cache = {}


def _plan(chunks_gb):
    """Shared host/builder structure. chunks_gb: (N_GROUPS, N_BANKS) ints.

    Returns dict with bank chunk streams and mappings:
      - chunk_of[(g, b, j)] -> global chunk index (meta column)
      - bank_ops[b] -> list of (global_chunk_start, n_chunks, slot_start)
      - total_chunks, total_slots
    """
    chunks_gb = np.asarray(chunks_gb)
    bank_chunks = chunks_gb.sum(axis=0)  # chunks per bank
    total_chunks = int(bank_chunks.sum())
    # global chunk order: bank-major, then group
    chunk_of = {}
    c = 0
    bank_first_chunk = []
    for b in range(N_BANKS):
        bank_first_chunk.append(c)
        for g in range(N_GROUPS):
            for j in range(chunks_gb[g, b]):
                chunk_of[(g, b, j)] = c
                c += 1
    assert c == total_chunks
    bank_ops = []
    for b in range(N_BANKS):
        ops = []
        done = 0
        while done < bank_chunks[b]:
            n = min(OP_CHUNKS, int(bank_chunks[b]) - done)
            c0 = bank_first_chunk[b] + done
            ops.append((c0, n, c0 * P))
            done += n
        bank_ops.append(ops)
    return {
        "chunks_gb": chunks_gb,
        "chunk_of": chunk_of,
        "bank_ops": bank_ops,
        "total_chunks": total_chunks,
        "total_slots": total_chunks * P,
    }


def _build_kernel(chunks_gb_key):
    plan = _plan(np.asarray(chunks_gb_key).reshape(N_GROUPS, N_BANKS))
    chunks_gb = plan["chunks_gb"]
    total_chunks = plan["total_chunks"]
    total_slots = plan["total_slots"]

    nc = bacc.Bacc("TRN2", target_bir_lowering=False, debug=False,
                   num_devices=N_CORES, num_swdge_queues=4)
    f32, f16 = mybir.dt.float32, mybir.dt.float16
    x_d = nc.dram_tensor("x", [N_NODES, N_FEAT], f16,
                         kind="ExternalInput").ap()
    bank_slots = [sum(plan["bank_ops"][b][i][1] * P
                      for i in range(len(plan["bank_ops"][b])))
                  for b in range(N_BANKS)]
    idx_ds = [nc.dram_tensor(f"midx{b}", [P, max(bank_slots[b] // 16, 1)],
                             mybir.dt.int16, kind="ExternalInput").ap()
              for b in range(N_BANKS)]
    slot_d = nc.dram_tensor("mslot", [P, total_chunks], f16,
                            kind="ExternalInput").ap()
    invd_d = nc.dram_tensor("minvd", [P, N_GROUPS], f32,
                            kind="ExternalInput").ap()
    iota_d = nc.dram_tensor("miota", [P, OP_CHUNKS * P], f16,
                            kind="ExternalInput").ap()
    out_d = nc.dram_tensor("out", [NODES_PER_CORE, N_FEAT], f32,
                           kind="ExternalOutput").ap()

    with tile.TileContext(nc) as tc, ExitStack() as ctx:
        meta_pool = ctx.enter_context(tc.tile_pool(name="meta", bufs=1))
        # Load the first gather op's indices (and the small sel-build
        # metadata) in tiny head DMAs so the pipeline starts immediately;
        # the bulky remainders follow and complete under the first waves.
        HEAD = OP_CHUNKS * P // 16  # idx cols covering one gather op
        idx_ts = []
        for b in range(N_BANKS):
            cols = max(bank_slots[b] // 16, 1)
            t = meta_pool.tile([P, cols], mybir.dt.int16, tag=f"idx{b}")
            h = min(HEAD, cols)
            nc.sync.dma_start(out=t[:, :h], in_=idx_ds[b][:, :h])
            idx_ts.append(t)
        iota_t = meta_pool.tile([P, OP_CHUNKS * P], f16)
        nc.sync.dma_start(out=iota_t[:], in_=iota_d[:])
        slot_t = meta_pool.tile([P, total_chunks], f16)
        sh = min(4 * OP_CHUNKS, total_chunks)
        nc.sync.dma_start(out=slot_t[:, :sh], in_=slot_d[:, :sh])
        invd_t = meta_pool.tile([P, N_GROUPS], f32)
        nc.sync.dma_start(out=invd_t[:], in_=invd_d[:])
        for b in range(N_BANKS):
            cols = max(bank_slots[b] // 16, 1)
            if cols > HEAD:
                nc.sync.dma_start(out=idx_ts[b][:, HEAD:],
                                  in_=idx_ds[b][:, HEAD:])
        if total_chunks > sh:
            nc.sync.dma_start(out=slot_t[:, sh:], in_=slot_d[:, sh:])

        gat_pool = ctx.enter_context(tc.tile_pool(name="gat", bufs=24))
        sel_pool = ctx.enter_context(tc.tile_pool(name="sel", bufs=24))
        psum_pool = ctx.enter_context(
            tc.tile_pool(name="psum", bufs=8, space="PSUM"))
        out_pool = ctx.enter_context(tc.tile_pool(name="outb", bufs=6))

        chunk_loc = {}  # global chunk idx -> (gather tile, block, sel tile)
        next_op = [0] * N_BANKS
        emitted_chunks = [0] * N_BANKS

        def emit_ops_until(b, need_chunks):
            """Emit gather ops on bank b until `need_chunks` chunks of its
            stream are available."""
            while emitted_chunks[b] < need_chunks:
                c0, n, s0 = plan["bank_ops"][b][next_op[b]]
                g_t = gat_pool.tile([P, OP_CHUNKS, N_FEAT], f16, tag="gat")
                sb = s0 - plan["bank_ops"][b][0][2]
                nc.gpsimd.dma_gather(
                    out_ap=g_t[:, :n, :],
                    in_ap=x_d[b * BANK:(b + 1) * BANK, :],
                    idxs_ap=idx_ts[b][:, sb // 16:(sb + n * P) // 16],
                    num_idxs=n * P,
                    num_idxs_reg=n * P,
                    elem_size=N_FEAT,
                    queue_num=b,
                    single_packet=True,
                )
                s_t = sel_pool.tile([P, OP_CHUNKS * P], f16, tag="sel")
                nc.vector.tensor_tensor(
                    out=s_t[:, :n * P],
                    in0=slot_t[:, c0:c0 + n].unsqueeze(2)
                        .to_broadcast([P, n, P]),
                    in1=iota_t[:, :n * P].rearrange("p (a b) -> p a b", a=n),
                    op=mybir.AluOpType.is_equal,
                )
                for j in range(n):
                    chunk_loc[c0 + j] = (g_t, j, s_t)
                next_op[b] += 1
                emitted_chunks[b] += n

        # per-bank running chunk counts per group (prefix sums)
        prefix = np.concatenate(
            [np.zeros((1, N_BANKS), int), np.cumsum(chunks_gb, axis=0)], axis=0)

        for g in range(N_GROUPS):
            nch_g = int(chunks_gb[g].sum())
            assert nch_g > 0
            for b in range(N_BANKS):
                emit_ops_until(b, int(prefix[g + 1, b]))
            ps = psum_pool.tile([P, N_FEAT], f32)
            i = 0
            for b in range(N_BANKS):
                for j in range(int(chunks_gb[g, b])):
                    c = plan["chunk_of"][(g, b, j)]
                    g_t, blk, s_t = chunk_loc.pop(c)
                    nc.tensor.matmul(
                        ps[:],
                        lhsT=s_t[:, blk * P:(blk + 1) * P],
                        rhs=g_t[:, blk, :],
                        start=(i == 0),
                        stop=(i == nch_g - 1),
                    )
                    i += 1
            o_t = out_pool.tile([P, N_FEAT], f32)
            nc.scalar.activation(out=o_t[:], in_=ps[:],
                                 func=mybir.ActivationFunctionType.Copy,
                                 scale=invd_t[:, g:g + 1])
            rows = min(P, NODES_PER_CORE - g * P)
            nc.sync.dma_start(out=out_d[g * P:g * P + rows, :],
                              in_=o_t[:rows, :])
    nc.compile()
    return nc


def _prepare(x, edge_src, edge_dst):
    x16 = np.ascontiguousarray(np.asarray(x), dtype=np.float16)
    src = np.asarray(edge_src).astype(np.int64)
    dst = np.asarray(edge_dst).astype(np.int64)

    deg = np.bincount(dst, minlength=N_NODES)
    inv_deg = (1.0 / np.maximum(deg, 1)).astype(np.float32)

    order = np.argsort(dst, kind="stable")
    src_s = src[order].astype(np.int32)
    dst_s = dst[order].astype(np.int32)
    bank_s = src_s // BANK

    # per (core, group, bank) counts
    cnt = np.zeros((N_CORES, N_GROUPS, N_BANKS), np.int64)
    core_s = dst_s // NODES_PER_CORE
    grp_s = (dst_s % NODES_PER_CORE) // P
    np.add.at(cnt, (core_s, grp_s, bank_s), 1)

    chunks_gb = -(-cnt.max(axis=0) // P)  # (N_GROUPS, N_BANKS)
    # ensure every group has at least one chunk so its PSUM/output is written
    empty = chunks_gb.sum(axis=1) == 0
    chunks_gb[empty, 0] = 1

    plan = _plan(chunks_gb)
    total_chunks = plan["total_chunks"]
    total_slots = plan["total_slots"]

    # slot offsets for cell (g, b) within the global meta arrays
    cell_start = {(g, b): plan["chunk_of"][(g, b, 0)] * P
                  for g in range(N_GROUPS) for b in range(N_BANKS)
                  if chunks_gb[g, b] > 0}

    in_maps = []
    for k in range(N_CORES):
        m = core_s == k
        ksrc, kdst, kbank, kgrp = src_s[m], dst_s[m], bank_s[m], grp_s[m]
        msrc = np.zeros((total_slots,), np.int16)
        mslot = np.full((total_slots,), -1.0, np.float16)
        # order edges by (bank, group) to match cell layout
        cell_id = kbank.astype(np.int64) * N_GROUPS + kgrp
        eorder = np.lexsort((ksrc, cell_id))
        ksrc, kdst, kbank, kgrp = (ksrc[eorder], kdst[eorder],
                                   kbank[eorder], kgrp[eorder])
        cid = kbank.astype(np.int64) * N_GROUPS + kgrp
        uniq, starts, counts = np.unique(cid, return_index=True,
                                         return_counts=True)
        for u, st, n in zip(uniq, starts, counts):
            b, g = int(u) // N_GROUPS, int(u) % N_GROUPS
            base = cell_start[(g, b)]
            assert n <= chunks_gb[g, b] * P
            msrc[base:base + n] = (ksrc[st:st + n] % BANK).astype(np.int16)
            mslot[base:base + n] = (
                kdst[st:st + n] - (k * NODES_PER_CORE + g * P)
            ).astype(np.float16)
        invd = np.zeros((N_GROUPS * P,), np.float32)
        invd[:NODES_PER_CORE] = inv_deg[k * NODES_PER_CORE:
                                        (k + 1) * NODES_PER_CORE]
        # idx layout: slot i at [16*r + i%16, i//16] for r in 0..7,
        # one tensor per bank (bank streams are contiguous in msrc)
        bank_slot_counts = []
        off = 0
        bank_idx_tiles = {}
        for b in range(N_BANKS):
            nb = sum(n * P for (_, n, _) in plan["bank_ops"][b])
            blk = msrc[off:off + nb]
            off += nb
            bank_idx_tiles[f"midx{b}"] = np.ascontiguousarray(
                np.tile(blk.reshape(-1, 16).T, (8, 1)))
        iota = np.tile(np.arange(P, dtype=np.float16)[None, :], (P, OP_CHUNKS))
        in_maps.append({
            "x": x16,
            **bank_idx_tiles,
            "mslot": np.ascontiguousarray(
                mslot.reshape(total_chunks, P).T),
            "minvd": np.ascontiguousarray(invd.reshape(N_GROUPS, P).T),
            "miota": np.ascontiguousarray(iota),
        })
    return in_maps, tuple(int(v) for v in chunks_gb.ravel())


def kernel(x, edge_src, edge_dst, _trace=False):
    in_maps, key = _prepare(x, edge_src, edge_dst)
    nc = _compiled_cache.get(key)
    if nc is None:
        nc = _build_kernel(key)
        _compiled_cache[key] = nc
    res = run_bass_kernel_spmd(nc, in_maps, core_ids=list(range(N_CORES)),
                               trace=_trace)
    out = np.concatenate([res.results[k]["out"] for k in range(N_CORES)],
                         axis=0)
    if _trace:
        kernel.last_exec_time_ns = res.exec_time_ns
    return out


# revision 14
# speedup vs baseline: 1.0672x; 1.0672x over previous
"""Segment-mean GNN aggregation (MeanAggregator) on 8 TRN2 NeuronCores.

out[v] = mean over edges (u -> v) of x[u], zeros for isolated nodes.

Strategy: shard destination nodes across the 8 cores (12500 each) and
replicate x (stored fp16) in every core's DRAM. The host partitions edges
by dst owner, sorts by dst, and packs them into 128-edge chunks grouped
by 128-dst "groups". Because dma_gather (the fast SWDGE gather) takes
int16 indices, x is split into 4 banks of 25000 rows and each chunk's
edges come from a single bank; the per-(group, bank) chunk counts are
maxed over cores so one SPMD program fits all 8 cores.

Device pipeline per core:
  - dma_gather ops of up to 8 chunks (1024 indices) pull source rows into
    SBUF [128 edges x nch x 128 feat] fp16 tiles; the 4 banks ride the 4
    SWDGE queues so all four Q7 pairs generate descriptors concurrently.
  - VectorE builds an exact one-hot S[e, s] = (slot[e] == s) in fp16 per
    chunk (tensor_scalar is_equal against a constant iota row); padding
    slots are -1 and match nothing.
  - TensorE accumulates S.T @ E into PSUM [128 dst x 128 feat] per group.
  - ScalarE copies PSUM to SBUF scaled by fp32 1/max(deg,1) (activation
    with per-partition scale), and the rows are DMA'd to the output.
"""

import math
from contextlib import ExitStack

import numpy as np

import concourse.tile as tile
from concourse import bacc, mybir
from concourse.bass_utils import run_bass_kernel_spmd

N_NODES = 100000
N_FEAT = 128
N_CORES = 8
NODES_PER_CORE = N_NODES // N_CORES  # 12500
P = 128
N_GROUPS = math.ceil(NODES_PER_CORE / P)  # 98
N_BANKS = 4
BANK = N_NODES // N_BANKS  # 25000 rows per bank (int16-indexable)
OP_CHUNKS = 8  # chunks per dma_gather op (1024 indices; single-packet safe)

_compiled_cache = {}


def _plan(chunks_gb):
    """Shared host/builder structure. chunks_gb: (N_GROUPS, N_BANKS) ints.

    Returns dict with bank chunk streams and mappings:
      - chunk_of[(g, b, j)] -> global chunk index (meta column)
      - bank_ops[b] -> list of (global_chunk_start, n_chunks, slot_start)
      - total_chunks, total_slots
    """
    chunks_gb = np.asarray(chunks_gb)
    bank_chunks = chunks_gb.sum(axis=0)  # chunks per bank
    total_chunks = int(bank_chunks.sum())
    # global chunk order: bank-major, then group
    chunk_of = {}
    c = 0
    bank_first_chunk = []
    for b in range(N_BANKS):
        bank_first_chunk.append(c)
        for g in range(N_GROUPS):
            for j in range(chunks_gb[g, b]):
                chunk_of[(g, b, j)] = c
                c += 1
    assert c == total_chunks
    bank_ops = []
    for b in range(N_BANKS):
        ops = []
        done = 0
        while done < bank_chunks[b]:
            n = min(OP_CHUNKS, int(bank_chunks[b]) - done)
            c0 = bank_first_chunk[b] + done
            ops.append((c0, n, c0 * P))
            done += n
        bank_ops.append(ops)
    return {
        "chunks_gb": chunks_gb,
        "chunk_of": chunk_of,
        "bank_ops": bank_ops,
        "total_chunks": total_chunks,
        "total_slots": total_chunks * P,
    }


def _build_kernel(chunks_gb_key):
    plan = _plan(np.asarray(chunks_gb_key).reshape(N_GROUPS, N_BANKS))
    chunks_gb = plan["chunks_gb"]
    total_chunks = plan["total_chunks"]
    total_slots = plan["total_slots"]

    nc = bacc.Bacc("TRN2", target_bir_lowering=False, debug=False,
                   num_devices=N_CORES, num_swdge_queues=4)
    f32, f16 = mybir.dt.float32, mybir.dt.float16
    x_d = nc.dram_tensor("x", [N_NODES, N_FEAT], f16,
                         kind="ExternalInput").ap()
    bank_slots = [sum(plan["bank_ops"][b][i][1] * P
                      for i in range(len(plan["bank_ops"][b])))
                  for b in range(N_BANKS)]
    idx_ds = [nc.dram_tensor(f"midx{b}", [P, max(bank_slots[b] // 16, 1)],
                             mybir.dt.int16, kind="ExternalInput").ap()
              for b in range(N_BANKS)]
    slot_d = nc.dram_tensor("mslot", [P, total_chunks], f16,
                            kind="ExternalInput").ap()
    invd_d = nc.dram_tensor("minvd", [P, N_GROUPS], f32,
                            kind="ExternalInput").ap()
    iota_d = nc.dram_tensor("miota", [P, OP_CHUNKS * P], f16,
                            kind="ExternalInput").ap()
    out_d = nc.dram_tensor("out", [NODES_PER_CORE, N_FEAT], f32,
                           kind="ExternalOutput").ap()

    with tile.TileContext(nc) as tc, ExitStack() as ctx:
        meta_pool = ctx.enter_context(tc.tile_pool(name="meta", bufs=1))
        idx_ts = []
        for b in range(N_BANKS):
            t = meta_pool.tile([P, max(bank_slots[b] // 16, 1)],
                               mybir.dt.int16, tag=f"idx{b}")
            nc.sync.dma_start(out=t[:], in_=idx_ds[b][:])
            idx_ts.append(t)
        slot_t = meta_pool.tile([P, total_chunks], f16)
        nc.sync.dma_start(out=slot_t[:], in_=slot_d[:])
        invd_t = meta_pool.tile([P, N_GROUPS], f32)
        nc.sync.dma_start(out=invd_t[:], in_=invd_d[:])
        iota_t = meta_pool.tile([P, OP_CHUNKS * P], f16)
        nc.sync.dma_start(out=iota_t[:], in_=iota_d[:])


        gat_pool = ctx.enter_context(tc.tile_pool(name="gat", bufs=24))
        sel_pool = ctx.enter_context(tc.tile_pool(name="sel", bufs=24))
        psum_pool = ctx.enter_context(
            tc.tile_pool(name="psum", bufs=8, space="PSUM"))
        out_pool = ctx.enter_context(tc.tile_pool(name="outb", bufs=6))

        chunk_loc = {}  # global chunk idx -> (gather tile, block, sel tile)
        next_op = [0] * N_BANKS
        emitted_chunks = [0] * N_BANKS

        def emit_ops_until(b, need_chunks):
            """Emit gather ops on bank b until `need_chunks` chunks of its
            stream are available."""
            while emitted_chunks[b] < need_chunks:
                c0, n, s0 = plan["bank_ops"][b][next_op[b]]
                g_t = gat_pool.tile([P, OP_CHUNKS, N_FEAT], f16, tag="gat")
                sb = s0 - plan["bank_ops"][b][0][2]
                nc.gpsimd.dma_gather(
                    out_ap=g_t[:, :n, :],
                    in_ap=x_d[b * BANK:(b + 1) * BANK, :],
                    idxs_ap=idx_ts[b][:, sb // 16:(sb + n * P) // 16],
                    num_idxs=n * P,
                    num_idxs_reg=n * P,
                    elem_size=N_FEAT,
                    queue_num=b,
                    single_packet=True,
                )
                s_t = sel_pool.tile([P, OP_CHUNKS * P], f16, tag="sel")
                nc.vector.tensor_tensor(
                    out=s_t[:, :n * P],
                    in0=slot_t[:, c0:c0 + n].unsqueeze(2)
                        .to_broadcast([P, n, P]),
                    in1=iota_t[:, :n * P].rearrange("p (a b) -> p a b", a=n),
                    op=mybir.AluOpType.is_equal,
                )
                for j in range(n):
                    chunk_loc[c0 + j] = (g_t, j, s_t)
                next_op[b] += 1
                emitted_chunks[b] += n

        # per-bank running chunk counts per group (prefix sums)
        prefix = np.concatenate(
            [np.zeros((1, N_BANKS), int), np.cumsum(chunks_gb, axis=0)], axis=0)

        for g in range(N_GROUPS):
            nch_g = int(chunks_gb[g].sum())
            assert nch_g > 0
            for b in range(N_BANKS):
                emit_ops_until(b, int(prefix[g + 1, b]))
            ps = psum_pool.tile([P, N_FEAT], f32)
            i = 0
            for b in range(N_BANKS):
                for j in range(int(chunks_gb[g, b])):
                    c = plan["chunk_of"][(g, b, j)]
                    g_t, blk, s_t = chunk_loc.pop(c)
                    nc.tensor.matmul(
                        ps[:],
                        lhsT=s_t[:, blk * P:(blk + 1) * P],
                        rhs=g_t[:, blk, :],
                        start=(i == 0),
                        stop=(i == nch_g - 1),
                    )
                    i += 1
            o_t = out_pool.tile([P, N_FEAT], f32)
            nc.scalar.activation(out=o_t[:], in_=ps[:],
                                 func=mybir.ActivationFunctionType.Copy,
                                 scale=invd_t[:, g:g + 1])
            rows = min(P, NODES_PER_CORE - g * P)
            nc.sync.dma_start(out=out_d[g * P:g * P + rows, :],
                              in_=o_t[:rows, :])
    nc.compile()
    return nc


def _prepare(x, edge_src, edge_dst):
    x16 = np.ascontiguousarray(np.asarray(x), dtype=np.float16)
    src = np.asarray(edge_src).astype(np.int64)
    dst = np.asarray(edge_dst).astype(np.int64)

    deg = np.bincount(dst, minlength=N_NODES)
    inv_deg = (1.0 / np.maximum(deg, 1)).astype(np.float32)

    order = np.argsort(dst, kind="stable")
    src_s = src[order].astype(np.int32)
    dst_s = dst[order].astype(np.int32)
    bank_s = src_s // BANK

    # per (core, group, bank) counts
    cnt = np.zeros((N_CORES, N_GROUPS, N_BANKS), np.int64)
    core_s = dst_s // NODES_PER_CORE
    grp_s = (dst_s % NODES_PER_CORE) // P
    np.add.at(cnt, (core_s, grp_s, bank_s), 1)

    chunks_gb = -(-cnt.max(axis=0) // P)  # (N_GROUPS, N_BANKS)
    # ensure every group has at least one chunk so its PSUM/output is written
    empty = chunks_gb.sum(axis=1) == 0
    chunks_gb[empty, 0] = 1

    plan = _plan(chunks_gb)
    total_chunks = plan["total_chunks"]
    total_slots = plan["total_slots"]

    # slot offsets for cell (g, b) within the global meta arrays
    cell_start = {(g, b): plan["chunk_of"][(g, b, 0)] * P
                  for g in range(N_GROUPS) for b in range(N_BANKS)
                  if chunks_gb[g, b] > 0}

    in_maps = []
    for k in range(N_CORES):
        m = core_s == k
        ksrc, kdst, kbank, kgrp = src_s[m], dst_s[m], bank_s[m], grp_s[m]
        msrc = np.zeros((total_slots,), np.int16)
        mslot = np.full((total_slots,), -1.0, np.float16)
        # order edges by (bank, group) to match cell layout
        cell_id = kbank.astype(np.int64) * N_GROUPS + kgrp
        eorder = np.lexsort((ksrc, cell_id))
        ksrc, kdst, kbank, kgrp = (ksrc[eorder], kdst[eorder],
                                   kbank[eorder], kgrp[eorder])
        cid = kbank.astype(np.int64) * N_GROUPS + kgrp
        uniq, starts, counts = np.unique(cid, return_index=True,
                                         return_counts=True)
        for u, st, n in zip(uniq, starts, counts):
            b, g = int(u) // N_GROUPS, int(u) % N_GROUPS
            base = cell_start[(g, b)]
            assert n <= chunks_gb[g, b] * P
            msrc[base:base + n] = (ksrc[st:st + n] % BANK).astype(np.int16)
            mslot[base:base + n] = (
                kdst[st:st + n] - (k * NODES_PER_CORE + g * P)
            ).astype(np.float16)
        invd = np.zeros((N_GROUPS * P,), np.float32)
        invd[:NODES_PER_CORE] = inv_deg[k * NODES_PER_CORE:
                                        (k + 1) * NODES_PER_CORE]
        # idx layout: slot i at [16*r + i%16, i//16] for r in 0..7,
        # one tensor per bank (bank streams are contiguous in msrc)
        bank_slot_counts = []
        off = 0
        bank_idx_tiles = {}
        for b in range(N_BANKS):
            nb = sum(n * P for (_, n, _) in plan["bank_ops"][b])
            blk = msrc[off:off + nb]
            off += nb
            bank_idx_tiles[f"midx{b}"] = np.ascontiguousarray(
                np.tile(blk.reshape(-1, 16).T, (8, 1)))
        iota = np.tile(np.arange(P, dtype=np.float16)[None, :], (P, OP_CHUNKS))
        in_maps.append({
            "x": x16,
            **bank_idx_tiles,
            "mslot": np.ascontiguousarray(
                mslot.reshape(total_chunks, P).T),
            "minvd": np.ascontiguousarray(invd.reshape(N_GROUPS, P).T),
            "miota": np.ascontiguousarray(iota),
        })
    return in_maps, tuple(int(v) for v in chunks_gb.ravel())


def kernel(x, edge_src, edge_dst, _trace=False):
    in_maps, key = _prepare(x, edge_src, edge_dst)
    nc = _compiled_cache.get(key)
    if nc is None:
        nc = _build_kernel(key)
        _compiled_cache[key] = nc
    res = run_bass_kernel_spmd(nc, in_maps, core_ids=list(range(N_CORES)),
                               trace=_trace)
    out = np.concatenate([res.results[k]["out"] for k in range(N_CORES)],
                         axis=0)
    if _trace:
        kernel.last_exec_time_ns = res.exec_time_ns
    return out
